# Initial kernel scaffold
#
"""Segment-mean (GNN mean-encoder) Trainium2 kernel.

Strategy (per the node-sharding variant of the sharding hint):
  * Host: partition nodes across the 8 cores round-robin in degree-sorted
    order, and repack the edge features into a jagged-diagonal (JDS) layout:
    slot j holds the j-th edge of every node that has > j edges.  Nodes are
    ranked by in-degree (descending), so slot j covers a contiguous prefix
    of ranks and the whole per-core tensor becomes one dense
    [128, SumB*D] f32 array (rank r -> partition r%128, block r//128),
    padded only up to 128-row slot boundaries (~1.5% overhead).
  * Device (one SPMD program on 8 NeuronCores): stream the dense array in
    ~2 MB column tiles, accumulate each slot's segment into a persistent
    [128, B*D] accumulator with DVE adds (all perfectly aligned, no
    indices needed), compute per-node counts by summing a 0/1 indicator
    array (the device-side equivalent of segment_sum(ones)), then multiply
    by the reciprocal and DMA the result out.
  * Host: inverse-permute the per-core outputs back to node order.

No cross-core communication is needed: each core owns a disjoint node set.
"""

import numpy as np
import ml_dtypes

import concourse.bass as bass
import concourse.tile as tile
from concourse import mybir
from concourse.bass_utils import run_bass_kernel_spmd

P = 128          # SBUF partitions
NCORES = 8
D = 32           # feature dim
N = 100000       # nodes
E = 1600000      # edges
CHUNK_BLOCKS = 128   # 128-byte blocks per streamed DMA tile -> 2 MiB DMAs

# test-harness hooks (the grading harness just calls kernel())
TRACE = False
TRACE_KWARGS = {}
LAST_RESULT = None


def _preprocess(e, dst):
    """Build per-core JDS arrays + indicator and the inverse permutation."""
    counts = np.bincount(dst, minlength=N)
    maxdeg = int(counts.max())
    order = np.argsort(-counts, kind="stable")          # nodes, degree desc
    inv = np.empty(N, np.int64)
    inv[order] = np.arange(N)
    core_of = inv % NCORES
    rank_of = inv // NCORES
    m = N // NCORES                                      # nodes per core
    B = (m + P - 1) // P                                 # accumulator blocks

    counts_sorted = counts[order]
    L = np.zeros((NCORES, maxdeg), np.int64)             # slot lengths
    for c in range(NCORES):
        cc = counts_sorted[c::NCORES]
        hist = np.bincount(cc, minlength=maxdeg + 1)
        L[c, :] = m - np.cumsum(hist)[:maxdeg]
    Bj = np.max((L + P - 1) // P, axis=0)                # blocks per slot
    Cj = np.concatenate([[0], np.cumsum(Bj)]).astype(np.int64)
    SumB = int(Cj[-1])

    # per-edge slot index = occurrence index within its dst group
    perm = np.argsort(dst, kind="stable")
    sd = dst[perm]
    newgrp = np.r_[True, sd[1:] != sd[:-1]]
    starts = np.flatnonzero(newgrp)
    group_id = np.cumsum(newgrp.astype(np.int64)) - 1
    j_e = np.arange(E, dtype=np.int64) - starts[group_id]

    c_e = core_of[sd]
    r_e = rank_of[sd]
    flat_idx = (r_e % P) * SumB + Cj[j_e] + (r_e // P)   # row in [P*SumB, D]

    e_jds = np.zeros((NCORES, P * SumB, D), np.float32)
    for c in range(NCORES):
        mask = c_e == c
        e_jds[c, flat_idx[mask]] = e[perm[mask]]

    # indicator [c, P, maxdeg*B] (bf16): 1 where a real (non-pad) node slot
    ranks = np.arange(B, dtype=np.int64)[None, :] * P + \
        np.arange(P, dtype=np.int64)[:, None]            # [P, B]
    ind = (ranks[None, None, :, :] < L[:, :, None, None])   # [c, j, P, B]
    ind = np.ascontiguousarray(
        ind.transpose(0, 2, 1, 3).reshape(NCORES, P, maxdeg * B)
    ).astype(ml_dtypes.bfloat16)

    return e_jds, ind, order, Bj, Cj, SumB, maxdeg, B, m


def _build_program(SumB, Bj, Cj, maxdeg, B):
    nc = bass.Bass()
    f32 = mybir.dt.float32
    ejds = nc.dram_tensor("ejds", [P, SumB * D], f32, kind="ExternalInput")
    ind = nc.dram_tensor(
        "ind", [P, maxdeg * B], mybir.dt.bfloat16, kind="ExternalInput"
    )
    out = nc.dram_tensor("out", [P, B * D], f32, kind="ExternalOutput")

    with tile.TileContext(nc) as tc:
        with (
            tc.tile_pool(name="acc", bufs=1) as acc_pool,
            tc.tile_pool(name="small", bufs=1) as small_pool,
            tc.tile_pool(name="stream", bufs=4) as stream_pool,
        ):
            A = acc_pool.tile([P, B * D], f32)
            nc.gpsimd.memset(A[:], 0.0)

            # counts = sum over slots of the indicator (segment_sum of ones)
            ind_sb = small_pool.tile([P, maxdeg * B], mybir.dt.bfloat16)
            nc.sync.dma_start(ind_sb[:], ind[:])
            recip = small_pool.tile([P, B], f32)
            nc.vector.reduce_sum(
                recip[:, :, None],
                ind_sb[:].rearrange("p (j b) -> p b j", b=B),
                axis=mybir.AxisListType.X,
            )
            nc.vector.tensor_scalar_max(recip[:], recip[:], 1.0)
            nc.vector.reciprocal(recip[:], recip[:])

            # stream the JDS array; every slot-aligned segment adds into A
            nchunks = (SumB + CHUNK_BLOCKS - 1) // CHUNK_BLOCKS
            for t in range(nchunks):
                blk0 = t * CHUNK_BLOCKS
                blk1 = min(SumB, blk0 + CHUNK_BLOCKS)
                w = blk1 - blk0
                tl = stream_pool.tile([P, CHUNK_BLOCKS * D], f32, tag="stream")
                nc.sync.dma_start(tl[:, : w * D], ejds[:, blk0 * D: blk1 * D])
                j = int(np.searchsorted(Cj, blk0, side="right")) - 1
                while j < maxdeg and Cj[j] < blk1:
                    s0 = max(blk0, int(Cj[j]))
                    s1 = min(blk1, int(Cj[j + 1]))
                    if s1 > s0:
                        alo = (s0 - int(Cj[j])) * D
                        ahi = alo + (s1 - s0) * D
                        nc.vector.tensor_add(
                            A[:, alo:ahi],
                            A[:, alo:ahi],
                            tl[:, (s0 - blk0) * D: (s1 - blk0) * D],
                        )
                    j += 1

            # mean = sums * (1 / max(count, 1)), recip broadcast across D
            nc.vector.tensor_mul(
                A[:].rearrange("p (b d) -> p b d", d=D),
                A[:].rearrange("p (b d) -> p b d", d=D),
                recip[:, :, None].broadcast_to([P, B, D]),
            )
            nc.sync.dma_start(out[:], A[:])
    return nc


def kernel(e, dst, n_nodes):
    global LAST_RESULT
    e = np.ascontiguousarray(np.asarray(e), dtype=np.float32)
    dst = np.asarray(dst).astype(np.int64)
    assert int(n_nodes) == N and e.shape == (E, D) and dst.shape == (E,)

    e_jds, ind, order, Bj, Cj, SumB, maxdeg, B, m = _preprocess(e, dst)

    nc = _build_program(SumB, Bj, Cj, maxdeg, B)
    in_maps = [
        {"ejds": e_jds[c].reshape(P, SumB * D), "ind": ind[c]}
        for c in range(NCORES)
    ]
    res = run_bass_kernel_spmd(
        nc,
        in_maps,
        core_ids=list(range(NCORES)),
        trace=TRACE,
        **TRACE_KWARGS,
    )
    LAST_RESULT = res

    out_full = np.zeros((N, D), np.float32)
    ranks = np.arange(m, dtype=np.int64)
    for c in range(NCORES):
        A = np.asarray(res.results[c]["out"]).reshape(P, B, D)
        # rank r lives at [r % P, r // P]; rank r is node order[8r + c]
        vals = A.transpose(1, 0, 2).reshape(B * P, D)[:m]
        out_full[order[c + NCORES * ranks]] = vals
    return out_full


# revision 10
# speedup vs baseline: 11.7040x; 11.7040x over previous
"""Segment-mean (GNN mean-encoder) Trainium2 kernel.

Strategy (per the node-sharding variant of the sharding hint):
  * Host: partition nodes across the 8 cores round-robin in degree-sorted
    order, and repack the edge features into a jagged-diagonal (JDS) layout:
    slot j holds the j-th edge of every node that has > j edges.  Nodes are
    ranked by in-degree (descending), so slot j covers a contiguous prefix
    of ranks and the whole per-core tensor becomes one dense
    [128, SumB*D] f32 array (rank r -> partition r%128, block r//128),
    padded only up to 128-row slot boundaries (~1.5% overhead).
  * Device (one SPMD program on 8 NeuronCores): stream the dense array in
    ~2 MB column tiles, accumulate each slot's segment into a persistent
    [128, B*D] accumulator with DVE adds (all perfectly aligned, no
    indices needed), compute per-node counts by summing a 0/1 indicator
    array (the device-side equivalent of segment_sum(ones)), then multiply
    by the reciprocal and DMA the result out.
  * Host: inverse-permute the per-core outputs back to node order.

No cross-core communication is needed: each core owns a disjoint node set.
"""

import numpy as np
import ml_dtypes

import concourse.bass as bass
import concourse.tile as tile
from concourse import mybir
from concourse.bass_utils import run_bass_kernel_spmd

P = 128          # SBUF partitions
NCORES = 8
D = 32           # feature dim
N = 100000       # nodes
E = 1600000      # edges
CHUNK_BLOCKS = 128   # 128-byte blocks per streamed DMA tile -> 2 MiB DMAs

# test-harness hooks (the grading harness just calls kernel())
TRACE = False
TRACE_KWARGS = {}
LAST_RESULT = None


def _preprocess(e, dst):
    """Build per-core JDS arrays + indicator and the inverse permutation."""
    counts = np.bincount(dst, minlength=N)
    maxdeg = int(counts.max())
    order = np.argsort(-counts, kind="stable")          # nodes, degree desc
    inv = np.empty(N, np.int64)
    inv[order] = np.arange(N)
    core_of = inv % NCORES
    rank_of = inv // NCORES
    m = N // NCORES                                      # nodes per core
    B = (m + P - 1) // P                                 # accumulator blocks

    counts_sorted = counts[order]
    L = np.zeros((NCORES, maxdeg), np.int64)             # slot lengths
    for c in range(NCORES):
        cc = counts_sorted[c::NCORES]
        hist = np.bincount(cc, minlength=maxdeg + 1)
        L[c, :] = m - np.cumsum(hist)[:maxdeg]
    Bj = np.max((L + P - 1) // P, axis=0)                # blocks per slot
    Cj = np.concatenate([[0], np.cumsum(Bj)]).astype(np.int64)
    SumB = int(Cj[-1])

    # per-edge slot index = occurrence index within its dst group
    perm = np.argsort(dst, kind="stable")
    sd = dst[perm]
    newgrp = np.r_[True, sd[1:] != sd[:-1]]
    starts = np.flatnonzero(newgrp)
    group_id = np.cumsum(newgrp.astype(np.int64)) - 1
    j_e = np.arange(E, dtype=np.int64) - starts[group_id]

    c_e = core_of[sd]
    r_e = rank_of[sd]
    flat_idx = (r_e % P) * SumB + Cj[j_e] + (r_e // P)   # row in [P*SumB, D]

    e_jds = np.zeros((NCORES, P * SumB, D), np.float32)
    for c in range(NCORES):
        mask = c_e == c
        e_jds[c, flat_idx[mask]] = e[perm[mask]]

    # indicator [c, P, maxdeg*B] (bf16): 1 where a real (non-pad) node slot
    ranks = np.arange(B, dtype=np.int64)[None, :] * P + \
        np.arange(P, dtype=np.int64)[:, None]            # [P, B]
    ind = (ranks[None, None, :, :] < L[:, :, None, None])   # [c, j, P, B]
    ind = np.ascontiguousarray(
        ind.transpose(0, 2, 1, 3).reshape(NCORES, P, maxdeg * B)
    ).astype(ml_dtypes.bfloat16)

    return e_jds, ind, order, Bj, Cj, SumB, maxdeg, B, m


def _split_multi_waits(nc):
    """Walrus in this toolchain rejects instructions with more than one sem
    wait ("Too many sync wait commands").  Tile's wait assignment is not
    transitively minimal, so e.g. a DMA reusing a pool slot waits on both the
    consumer engine's sem and its own lane's previous DMA.  Hoist all but one
    wait of each instruction onto same-engine NoOps inserted right before it:
    the sequencer executes them in order, so semantics are identical.
    """
    ctr = 0
    for fn in nc.m.functions:
        for bb in fn.blocks:
            new_insts = []
            for inst in bb.instructions:
                si = inst.sync_info
                if si is not None and si.on_wait and len(si.on_wait) > 1:
                    waits = list(si.on_wait)
                    for w in waits[:-1]:
                        ctr += 1
                        nop = mybir.InstNoOp(
                            name=f"I-waitsplit-{ctr}",
                            engine=inst.engine,
                            ins=[],
                            outs=[],
                            sync_info=mybir.SyncInfo(on_wait=[w], on_update=[]),
                        )
                        new_insts.append(nop)
                    si.on_wait = [waits[-1]]
                new_insts.append(inst)
            bb.instructions = new_insts


def _build_program(SumB, Bj, Cj, maxdeg, B, repeats=1):
    nc = bass.Bass()
    f32 = mybir.dt.float32
    ejds = nc.dram_tensor("ejds", [P, SumB * D], f32, kind="ExternalInput")
    ind = nc.dram_tensor(
        "ind", [P, maxdeg * B], mybir.dt.bfloat16, kind="ExternalInput"
    )
    out = nc.dram_tensor("out", [P, B * D], f32, kind="ExternalOutput")

    with tile.TileContext(nc) as tc:
        with (
            tc.tile_pool(name="acc", bufs=1) as acc_pool,
            tc.tile_pool(name="small", bufs=2) as small_pool,
            tc.tile_pool(name="stream", bufs=8) as stream_pool,
        ):
            A = acc_pool.tile([P, B * D], f32)
            for _rep in range(repeats):
                # memset on DVE: the first accumulate add then depends on it
                # via same-engine order (TT encoding fits only 1 sem wait)
                nc.vector.memset(A[:], 0.0)

                # counts = sum over slots of indicator (segment_sum of ones)
                ind_sb = small_pool.tile(
                    [P, maxdeg * B], mybir.dt.bfloat16, tag="ind_sb"
                )
                nc.sync.dma_start(ind_sb[:], ind[:])
                recip = small_pool.tile([P, B], f32, tag="recip")
                nc.vector.reduce_sum(
                    recip[:, :, None],
                    ind_sb[:].rearrange("p (j b) -> p b j", b=B),
                    axis=mybir.AxisListType.X,
                )
                nc.vector.tensor_scalar_max(recip[:], recip[:], 1.0)
                nc.vector.reciprocal(recip[:], recip[:])

                # stream the JDS array; each slot-aligned segment adds into A
                nchunks = (SumB + CHUNK_BLOCKS - 1) // CHUNK_BLOCKS
                for t in range(nchunks):
                    blk0 = t * CHUNK_BLOCKS
                    blk1 = min(SumB, blk0 + CHUNK_BLOCKS)
                    w = blk1 - blk0
                    tl = stream_pool.tile(
                        [P, CHUNK_BLOCKS * D], f32, tag="stream"
                    )
                    nc.sync.dma_start(
                        tl[:, : w * D], ejds[:, blk0 * D: blk1 * D]
                    )
                    j = int(np.searchsorted(Cj, blk0, side="right")) - 1
                    while j < maxdeg and Cj[j] < blk1:
                        s0 = max(blk0, int(Cj[j]))
                        s1 = min(blk1, int(Cj[j + 1]))
                        if s1 > s0:
                            alo = (s0 - int(Cj[j])) * D
                            ahi = alo + (s1 - s0) * D
                            nc.vector.tensor_add(
                                A[:, alo:ahi],
                                A[:, alo:ahi],
                                tl[:, (s0 - blk0) * D: (s1 - blk0) * D],
                            )
                        j += 1

                # mean = sums * (1 / max(count, 1)), recip broadcast over D
                nc.vector.tensor_mul(
                    A[:].rearrange("p (b d) -> p b d", d=D),
                    A[:].rearrange("p (b d) -> p b d", d=D),
                    recip[:, :, None].broadcast_to([P, B, D]),
                )
                nc.sync.dma_start(out[:], A[:])
    _split_multi_waits(nc)
    return nc


def _make_runner(nc, in_maps):
    """Build a repeat-callable PJRT runner with inputs staged on-device once.

    Mirrors bass2jax.run_bass_via_pjrt's multi-core path, minus output-buffer
    donation (so the staged arrays can be reused across timing calls).
    """
    import jax
    from jax.experimental.shard_map import shard_map
    from jax.sharding import Mesh, NamedSharding, PartitionSpec

    from concourse import bass2jax

    bass2jax.install_neuronx_cc_hook()
    n_cores = len(in_maps)

    partition_name = (
        nc.partition_id_tensor.name if nc.partition_id_tensor else None
    )
    in_names, out_names, out_avals, zero_outs = [], [], [], []
    for alloc in nc.m.functions[0].allocations:
        if not isinstance(alloc, mybir.MemoryLocationSet):
            continue
        name = alloc.memorylocations[0].name
        if alloc.kind == "ExternalInput":
            if name != partition_name:
                in_names.append(name)
        elif alloc.kind == "ExternalOutput":
            out_names.append(name)
            shape = tuple(alloc.tensor_shape)
            dtype = mybir.dt.np(alloc.dtype)
            out_avals.append(jax.core.ShapedArray(shape, dtype))
            zero_outs.append(np.zeros(shape, dtype))
    n_params = len(in_names)
    all_names = in_names + out_names
    if partition_name is not None:
        all_names = all_names + [partition_name]

    def _body(*args):
        operands = list(args)
        if partition_name is not None:
            operands.append(bass2jax.partition_id_tensor())
        outs = bass2jax._bass_exec_p.bind(
            *operands,
            out_avals=tuple(out_avals),
            in_names=tuple(all_names),
            out_names=tuple(out_names),
            lowering_input_output_aliases=(),
            sim_require_finite=True,
            sim_require_nnan=True,
            nc=nc,
        )
        return tuple(outs)

    devices = jax.devices()[:n_cores]
    mesh = Mesh(np.asarray(devices), ("core",))
    nmaps = n_params + len(out_names)
    sharded = jax.jit(
        shard_map(
            _body,
            mesh=mesh,
            in_specs=(PartitionSpec("core"),) * nmaps,
            out_specs=(PartitionSpec("core"),) * len(out_names),
            check_rep=False,
        ),
        keep_unused=True,
    )
    sh = NamedSharding(mesh, PartitionSpec("core"))
    staged = [
        jax.device_put(
            np.concatenate([np.asarray(m[name]) for m in in_maps], axis=0), sh
        )
        for name in in_names
    ] + [
        jax.device_put(
            np.zeros((n_cores * z.shape[0], *z.shape[1:]), z.dtype), sh
        )
        for z in zero_outs
    ]

    def run():
        outs = sharded(*staged)
        for o in outs:
            o.block_until_ready()
        return outs

    return run


def kernel(e, dst, n_nodes):
    global LAST_RESULT
    e = np.ascontiguousarray(np.asarray(e), dtype=np.float32)
    dst = np.asarray(dst).astype(np.int64)
    assert int(n_nodes) == N and e.shape == (E, D) and dst.shape == (E,)

    e_jds, ind, order, Bj, Cj, SumB, maxdeg, B, m = _preprocess(e, dst)

    nc = _build_program(SumB, Bj, Cj, maxdeg, B)
    in_maps = [
        {"ejds": e_jds[c].reshape(P, SumB * D), "ind": ind[c]}
        for c in range(NCORES)
    ]
    res = run_bass_kernel_spmd(
        nc,
        in_maps,
        core_ids=list(range(NCORES)),
        trace=TRACE,
        **TRACE_KWARGS,
    )
    LAST_RESULT = res

    out_full = np.zeros((N, D), np.float32)
    ranks = np.arange(m, dtype=np.int64)
    for c in range(NCORES):
        A = np.asarray(res.results[c]["out"]).reshape(P, B, D)
        # rank r lives at [r % P, r // P]; rank r is node order[8r + c]
        vals = A.transpose(1, 0, 2).reshape(B * P, D)[:m]
        out_full[order[c + NCORES * ranks]] = vals
    return out_full


def benchmark(e, dst, n_nodes, r_lo=4, r_hi=24, calls=8):
    """Estimate steady-state per-invocation HW time via the slope method:
    two programs with the kernel body repeated r_lo / r_hi times; the
    difference in min wall time isolates on-device time from RPC/staging
    overhead (inputs are staged on-device once per program).
    Returns (ns_per_invocation, details_dict)."""
    import time

    e = np.ascontiguousarray(np.asarray(e), dtype=np.float32)
    dst = np.asarray(dst).astype(np.int64)
    e_jds, ind, order, Bj, Cj, SumB, maxdeg, B, m = _preprocess(e, dst)
    in_maps = [
        {"ejds": e_jds[c].reshape(P, SumB * D), "ind": ind[c]}
        for c in range(NCORES)
    ]

    results = {}
    for R in (r_lo, r_hi):
        nc = _build_program(SumB, Bj, Cj, maxdeg, B, repeats=R)
        run = _make_runner(nc, in_maps)
        run()  # compile + warmup
        run()
        times = []
        for _ in range(calls):
            t0 = time.perf_counter()
            run()
            times.append(time.perf_counter() - t0)
        results[R] = times
        print(f"R={R}: times(ms) = {[f'{t*1e3:.2f}' for t in sorted(times)]}")

    tau = (min(results[r_hi]) - min(results[r_lo])) / (r_hi - r_lo)
    return tau * 1e9, results
